# revision 1
# baseline (speedup 1.0000x reference)
"""Binarized 3x3 conv (N=32, C=256->256, H=W=56, pad 1) on 8 TRN2 NeuronCores.

Sharding: data-parallel over batch (4 images per core), weights replicated.

Math: binarize exactly via
  xb = (x >= 0) - 0.5            in {+-0.5}  (exact in fp8 e4m3)
  wb = (w >= 0) - 0.5            in {+-0.5}  (exact in fp8 e4m3)
so every product is exactly +-0.25 and fp32 PSUM accumulation is exact
(quarter-integer partial sums, |.| <= 576 << 2^22). The output drain applies
scale=4.0 to restore the +-1-product conv result. sign(0)=+1 is honored.

Conv as matmul: the padded (58x58) binarized image lives flat in SBUF, so for
each kernel tap (kh,kw) the needed input window is a CONTIGUOUS span of the
flat padded grid shifted by (kh-1)*58+(kw-1). Outputs are computed on the
padded grid (464-wide spans = 8 padded rows) and the two garbage columns per
row (conv centered on pad columns) are dropped at drain time.

TensorE: fp8 DoubleRow matmuls contract all 256 input channels in one
instruction (K=128 partitions x 2 interleaved weights/cell), 9 accumulating
matmuls (one per tap) per output tile. 2 co-chunks x 4 images x 7 row-groups
x 9 taps = 504 matmuls per core.

Weights: ONE contiguous DMA loads w[o, i, kh, kw] as [o_local=128 part,
(oc, i, tap)] (256 descriptors of 9216B — the HBM-contiguous axis (i, tap)
lands on the SBUF free axis). The o<->i transpose needed for the matmul
lhsT layout [ci_local][two][co] is done on-chip: 36 PE transpose-mode
matmuls of 128x128 f32 blocks (strided columns, stride 9) into PSUM, each
drained by a DVE tensor_scalar that fuses the binarize to {+-0.5} fp8 and
scatters into the DoubleRow layout [tap][two][co]. This replaces the old
36B-run gather DMA (131072 descriptors, ~38 ms) with ~25 us of work.
"""

import os
os.environ.setdefault("CONCOURSE_SCRUB_NEFF_DEBUG_INFO", "1")

import numpy as np

import concourse.bass as bass
import concourse.mybir as mybir
import concourse.tile as tile
from concourse import bacc, bass_utils, masks

N_CORES = 8
N, CIN, H, W = 32, 256, 56, 56
COUT, KS = 256, 3
NPC = N // N_CORES          # images per core
HP, WP = H + 2, W + 2       # padded spatial (58x58)
GRID = HP * WP              # 3364
LEAD = 64                   # per-chunk front pad so tap offsets never go negative
CHUNK = 3440                # LEAD + GRID + 12 tail, %16 == 0 (DoubleRow step)
NROW_GROUPS = 7
ROWS_PER_GROUP = H // NROW_GROUPS   # 8
FREE = ROWS_PER_GROUP * WP          # 464 <= 512 (one PSUM bank, fp32)
CI_CHUNKS = CIN // 128
CO_CHUNKS = COUT // 128

F32 = mybir.dt.float32
FP8 = mybir.dt.float8e4
ALU = mybir.AluOpType
AF = mybir.ActivationFunctionType
DR = mybir.MatmulPerfMode.DoubleRow

# tap groups for the weight-transpose drains: 4+4+1 blocks per 512-f32 PSUM bank
TAP_GROUPS = [(0, 4), (4, 4), (8, 1)]


def _body(tc, x_d, w_d, b_d, o_d, repeats=1, parts="full"):
    nc = tc.nc

    from contextlib import ExitStack
    ctx = ExitStack()
    with ctx:
        const_pool = ctx.enter_context(tc.tile_pool(name="const", bufs=1))
        wd_pool = ctx.enter_context(tc.tile_pool(name="wd", bufs=1))
        wsb_pool = ctx.enter_context(tc.tile_pool(name="wsb", bufs=1))
        xpad_pool = ctx.enter_context(tc.tile_pool(name="xpad", bufs=1))
        xin_pool = ctx.enter_context(tc.tile_pool(name="xin", bufs=3))
        out_pool = ctx.enter_context(tc.tile_pool(name="outs", bufs=2))

        ident = const_pool.tile([128, 128], F32, tag="ident", name="ident")
        masks.make_identity(nc, ident[:])

        bias_sb = const_pool.tile([128, CO_CHUNKS], F32, tag="bias",
                                  name="bias_sb")

        o_d3 = [[o_d[n, cc * 128:(cc + 1) * 128].rearrange("c h w -> c (h w)")
                 for cc in range(CO_CHUNKS)] for n in range(NPC)]

        for rep in range(repeats):
            # ---- weight phase: contiguous DMAs + on-chip transpose ----
            # wsb[cc]: [o_local=128, (i, tap)] — HBM-contiguous (i, tap) on
            # the free axis, so this is 128 descriptors of 9216B per chunk.
            # One tile per co-chunk (so cc0's transposes depend only on cc0's
            # DMA), issued on the ACT HWDGE ring (nc.scalar) while the SP
            # ring streams x.
            w_src = w_d.rearrange("(oc p) i kh kw -> p oc (i kh kw)", p=128)
            wsb = []
            for cc in range(CO_CHUNKS):
                wt = wsb_pool.tile([128, CIN * KS * KS], F32,
                                   tag=f"wsb{cc}", name=f"wsb{rep}_{cc}")
                # halves by ci-chunk: the two=0 transposes only wait for the
                # first half, shortening the weight-phase critical chain
                half = 128 * KS * KS
                nc.scalar.dma_start(wt[:, :half], w_src[:, cc, :half])
                nc.scalar.dma_start(wt[:, half:], w_src[:, cc, half:])
                wsb.append(wt)
            if rep == 0:
                nc.scalar.dma_start(bias_sb[:],
                                    b_d.rearrange("(c p) -> p c", p=128))
            wviews = [t[:].rearrange("p (i t) -> p i t", t=KS * KS)
                      for t in wsb]

            # wd8[cc]: [128 ci_local, 9*256] fp8, free idx = tap*256 + two*128
            # + co, values (w>=0)-0.5 in {+-0.5}. (lhsT slice per tap:
            # [k][two][m], steps [128, 1] — DoubleRow pairing contracts
            # (k, two) elementwise on both operands.)
            wd8 = []
            for cc in range(CO_CHUNKS):
                wt = wd_pool.tile([128, KS * KS * 256], FP8, tag=f"wd{cc}",
                                  name=f"wd8_{rep}_{cc}")
                wd8.append(wt)

            xpall = xpad_pool.tile([128, NPC * CI_CHUNKS * CHUNK], FP8,
                                   tag="xpall", name=f"xpall{rep}")
            xg4 = xpall[:].rearrange("c (n t s) -> c n t s",
                                     t=CI_CHUNKS, s=CHUNK)
            hh = H // 2

            def emit_weight_cc(cc, wtpsum):
                wt3 = wd8[cc][:].rearrange("k (t x) -> k t x", t=KS * KS)
                for two in range(CI_CHUNKS):
                    for g, (t0, tn) in enumerate(TAP_GROUPS):
                        pt = wtpsum.tile([128, 512], F32, tag="wtp",
                                         name=f"wtp{rep}_{cc}_{two}_{g}")
                        for j in range(tn):
                            nc.tensor.transpose(
                                pt[:, j * 128:(j + 1) * 128],
                                wviews[cc][:, two * 128:(two + 1) * 128,
                                           t0 + j],
                                ident[:])
                        # drain + binarize: {+-0.5} fp8, scattered to
                        # [tap][two][co] (dst strides: tap 256, co 1)
                        nc.vector.tensor_scalar(
                            wt3[:, t0:t0 + tn, two * 128:(two + 1) * 128],
                            pt[:, :tn * 128].rearrange(
                                "k (t x) -> k t x", x=128),
                            0.0, 0.5, op0=ALU.is_ge, op1=ALU.subtract)

            def emit_memsets():
                # borders of all 8 padded grids zeroed with 6 multi-grid
                # strided memsets (disjoint from the binarize interiors,
                # so they run concurrently)
                xg = xpall[:].rearrange("c (g s) -> c g s", s=CHUNK)
                nc.gpsimd.memset(xg[:, :, 0:LEAD], 0.0)
                nc.gpsimd.memset(xg[:, :, LEAD + GRID:CHUNK], 0.0)
                xgrid = xg[:, :, LEAD:LEAD + GRID] \
                    .rearrange("c g (h w) -> c g h w", w=WP)
                nc.gpsimd.memset(xgrid[:, :, 0:1, :], 0.0)
                nc.gpsimd.memset(xgrid[:, :, HP - 1:HP, :], 0.0)
                nc.gpsimd.memset(xgrid[:, :, 1:HP - 1, 0:1], 0.0)
                nc.gpsimd.memset(xgrid[:, :, 1:HP - 1, WP - 1:WP], 0.0)

            def emit_input(n):
                # x loads and binarize split in row-halves, top halves of
                # both ci-chunks first: the first matmul group only reads
                # rows 0-9, so it starts after ~half of image 0 has landed.
                # Image 0 binarizes on the (idle-until-outputs) GPSIMD so
                # DVE's early queue only carries the weight drains.
                eng = nc.gpsimd if n == 0 else nc.vector
                xraws = [xin_pool.tile([128, H * W], F32, tag="xraw",
                                       name=f"xraw{rep}_{n}_{two}")
                         for two in range(CI_CHUNKS)]
                for h in range(2):
                    for two in range(CI_CHUNKS):
                        xr_in = xraws[two][:].rearrange("c (h w) -> c h w",
                                                        w=W)
                        nc.sync.dma_start(
                            xr_in[:, h * hh:(h + 1) * hh],
                            x_d[n, two * 128:(two + 1) * 128,
                                h * hh:(h + 1) * hh])
                        xg_in = xg4[:, n, two, LEAD:LEAD + GRID] \
                            .rearrange("c (h w) -> c h w", w=WP)
                        eng.tensor_scalar(
                            xg_in[:, 1 + h * hh:1 + (h + 1) * hh, 1:W + 1],
                            xr_in[:, h * hh:(h + 1) * hh],
                            0.0, 0.5, op0=ALU.is_ge, op1=ALU.subtract)

            # HAM warmup: the PE is idle until the first weight DMA lands
            # (~4us) and would then run its first ~3.4us of real work at
            # 1.2 GHz (cold clock-gate). Dummy identity matmuls during the
            # DMA wait release the throttle before the transposes start.
            # (Transpose-mode matmuls don't count as PE-busy for HAM.)
            if rep == 0:
                with tc.tile_pool(name="warm", bufs=1, space="PSUM") as wp:
                    warm = wp.tile([128, 128], F32, tag="warm", name="warm")
                    for i in range(8):
                        nc.tensor.matmul(warm[:], ident[:], ident[:],
                                         start=True, stop=True)

            # emission order = scheduling priority: cc0 weights, then the
            # first image, then cc1 weights, then the remaining images —
            # so DVE's early queue isn't blocked by cc1's weight drains
            # (which wait on the second weight DMA) ahead of image-0
            # binarize work
            n_inputs = NPC if parts not in ("mmonly", "mmraw", "mmraweven", "mmrawsame") else 0
            with tc.tile_pool(name="wtp", bufs=2, space="PSUM") as wtpsum:
                emit_weight_cc(0, wtpsum)
                emit_memsets()
                if n_inputs:
                    emit_input(0)
                emit_weight_cc(1, wtpsum)
                for n in range(1, n_inputs):
                    emit_input(n)
            xp = [xpall[:, n * CI_CHUNKS * CHUNK:(n + 1) * CI_CHUNKS * CHUNK]
                  for n in range(NPC)]

            # ---- conv phase ----
            # per-row-group PSUM tiles rotating through all 8 banks: group
            # g+1's first matmul into a bank only waits for a drain from
            # ~1.5 groups earlier, so TensorE never stalls on drains
            with tc.tile_pool(name="cpsum", bufs=8, space="PSUM") as cpsum:
                ngroups = NPC * CO_CHUNKS if parts != "nomm" else 0
                for gi in range(ngroups):
                    n, cc = divmod(gi, CO_CHUNKS)
                    pps = [cpsum.tile([128, 512], F32, tag="cps",
                                      name=f"cps{rep}_{cc}_{n}_{rg}")
                           for rg in range(NROW_GROUPS)]
                    # rg-outer / tap-inner: consecutive matmuls stream
                    # 98%-overlapping rhs spans (offsets +-1, +-58), hitting
                    # the HW span-reuse fast path (measured: re-streaming an
                    # identical span is ~1.4x faster than a fresh one; LDW
                    # is emitted per-matmul either way, so tap-inner loses
                    # nothing on the weight side)
                    for rg in range(NROW_GROUPS):
                        for kpos in range(KS * KS):
                            kh, kw = divmod(kpos, KS)
                            lhsT = wd8[cc][:, kpos * 256:(kpos + 1) * 256] \
                                .rearrange("k (two m) -> k two m", two=2)
                            off = (LEAD + WP + rg * FREE
                                   + (kh - 1) * WP + (kw - 1))
                            if parts == "mmraweven":
                                # timing-only probe: 16B-align every rhs
                                # span to test DR column-fetch alignment
                                off &= ~15
                            elif parts == "mmrawsame":
                                # timing-only probe: all 9 taps re-stream
                                # ONE identical (misaligned-base) span per
                                # rg — discriminates span-reuse vs
                                # alignment as the mmraweven fast path
                                off = LEAD + WP + rg * FREE
                            rhs = xp[n].rearrange(
                                "k (two s) -> k two s",
                                s=CHUNK)[:, :, off:off + FREE]
                            nc.tensor.matmul(
                                pps[rg][:, :FREE], lhsT,
                                rhs, start=(kpos == 0),
                                stop=(kpos == KS * KS - 1),
                                perf_mode=DR)
                    ob = out_pool.tile([128, NROW_GROUPS * ROWS_PER_GROUP * W],
                                       F32, tag="ob",
                                       name=f"ob{rep}_{cc}_{n}")
                    # per-row-group drains (x4 restores the +-0.25 products),
                    # alternating ACT/DVE so the serial drain tail halves
                    # (mmraw/nodrain ablations: 8-col token drains that still
                    # read every PSUM bank so DCE keeps the matmuls)
                    ncol = W if parts not in ("mmraw", "mmraweven", "mmrawsame", "nodrain") else 8
                    for rg in range(NROW_GROUPS):
                        drain_in = pps[rg][:, :FREE] \
                            .rearrange("m (r c) -> m r c", c=WP
                                       )[:, :, 1:ncol + 1]
                        drain_out = ob[:].rearrange(
                            "m (g r c) -> m g r c", g=NROW_GROUPS, c=W
                            )[:, rg, :, :ncol]
                        if rg % 2 == 0:
                            nc.scalar.activation(
                                drain_out, drain_in,
                                AF.Identity, bias=bias_sb[:, cc:cc + 1],
                                scale=4.0)
                        else:
                            nc.vector.tensor_scalar(
                                drain_out, drain_in,
                                4.0, bias_sb[:, cc:cc + 1],
                                op0=ALU.mult, op1=ALU.add)
                    # outputs ride the idle SWDGE/Pool path so they never
                    # contend with the SP ring streaming x; the last group
                    # is split so its early quarters overlap the final drains
                    ob_g = ob[:].rearrange("m (g s) -> m g s", g=NROW_GROUPS)
                    od_g = o_d3[n][cc].rearrange("c (g s) -> c g s",
                                                 g=NROW_GROUPS)
                    if parts in ("noout", "mmraw", "mmraweven", "mmrawsame", "nodrain"):
                        # tiny consumer keeps drains/MMs live through DCE
                        nc.gpsimd.dma_start(od_g[:, 0, :64], ob_g[:, 0, :64])
                    elif gi == ngroups - 1:
                        # both HWDGE rings are idle by now; alternate the
                        # quarters so the tail transfer time halves
                        for qi, (lo, hi) in enumerate(
                                ((0, 2), (2, 4), (4, 6), (6, 7))):
                            eng = nc.sync if qi % 2 == 0 else nc.scalar
                            eng.dma_start(od_g[:, lo:hi], ob_g[:, lo:hi])
                    else:
                        nc.gpsimd.dma_start(o_d3[n][cc], ob[:])


_nc_cache = {}


def _get_nc(repeats=1, parts="full"):
    key = (repeats, parts)
    if key not in _nc_cache:
        nc = bacc.Bacc("TRN2", debug=False)
        x_d = nc.dram_tensor("x", [NPC, CIN, H, W], F32, kind="ExternalInput").ap()
        w_d = nc.dram_tensor("w", [COUT, CIN, KS, KS], F32,
                             kind="ExternalInput").ap()
        b_d = nc.dram_tensor("b", [COUT], F32, kind="ExternalInput").ap()
        o_d = nc.dram_tensor("out", [NPC, COUT, H, W], F32,
                             kind="ExternalOutput").ap()
        with tile.TileContext(nc) as tc:
            _body(tc, x_d, w_d, b_d, o_d, repeats=repeats, parts=parts)
        nc.compile()
        _nc_cache[key] = nc
    return _nc_cache[key]


def _run(inputs, repeats=1, **kwargs):
    x, w, b = inputs["x"], inputs["w"], inputs["b"]
    assert x.shape == (N, CIN, H, W), x.shape
    nc = _get_nc(repeats)
    in_maps = [{
        "x": np.ascontiguousarray(x[i * NPC:(i + 1) * NPC], dtype=np.float32),
        "w": np.ascontiguousarray(w, dtype=np.float32),
        "b": np.ascontiguousarray(b, dtype=np.float32),
    } for i in range(N_CORES)]
    res = bass_utils.run_bass_kernel_spmd(
        nc, in_maps, core_ids=list(range(N_CORES)), **kwargs)
    out = np.concatenate([res.results[i]["out"] for i in range(N_CORES)], axis=0)
    return out, res


def kernel(**inputs) -> np.ndarray:
    out, _ = _run(inputs)
    return out



# revision 3
# speedup vs baseline: 829.2516x; 829.2516x over previous
"""Binarized 3x3 conv (N=32, C=256->256, H=W=56, pad 1) on 8 TRN2 NeuronCores.

Sharding: data-parallel over batch (4 images per core), weights replicated.

Math: binarize exactly via
  xb = (x >= 0) - 0.5            in {+-0.5}  (exact in fp8 e4m3)
  wb = (w >= 0) - 0.5            in {+-0.5}  (exact in fp8 e4m3)
so every product is exactly +-0.25 and fp32 PSUM accumulation is exact
(quarter-integer partial sums, |.| <= 576 << 2^22). The output drain applies
scale=4.0 to restore the +-1-product conv result. sign(0)=+1 is honored.

Conv as matmul: the padded (58x58) binarized image lives flat in SBUF, so for
each kernel tap (kh,kw) the needed input window is a CONTIGUOUS span of the
flat padded grid shifted by (kh-1)*58+(kw-1). Outputs are computed on the
padded grid (464-wide spans = 8 padded rows) and the two garbage columns per
row (conv centered on pad columns) are dropped at drain time.

TensorE: fp8 DoubleRow matmuls contract all 256 input channels in one
instruction (K=128 partitions x 2 interleaved weights/cell), 9 accumulating
matmuls (one per tap) per output tile. 2 co-chunks x 4 images x 7 row-groups
x 9 taps = 504 matmuls per core.

Weights: ONE contiguous DMA loads w[o, i, kh, kw] as [o_local=128 part,
(oc, i, tap)] (256 descriptors of 9216B — the HBM-contiguous axis (i, tap)
lands on the SBUF free axis). The o<->i transpose needed for the matmul
lhsT layout [ci_local][two][co] is done on-chip: 36 PE transpose-mode
matmuls of 128x128 f32 blocks (strided columns, stride 9) into PSUM, each
drained by a DVE tensor_scalar that fuses the binarize to {+-0.5} fp8 and
scatters into the DoubleRow layout [tap][two][co]. This replaces the old
36B-run gather DMA (131072 descriptors, ~38 ms) with ~25 us of work.
"""

import os
os.environ.setdefault("CONCOURSE_SCRUB_NEFF_DEBUG_INFO", "1")

import numpy as np

import concourse.bass as bass
import concourse.mybir as mybir
import concourse.tile as tile
from concourse import bacc, bass_utils, masks

N_CORES = 8
N, CIN, H, W = 32, 256, 56, 56
COUT, KS = 256, 3
NPC = N // N_CORES          # images per core
HP, WP = H + 2, W + 2       # padded spatial (58x58)
GRID = HP * WP              # 3364
LEAD = 64                   # per-chunk front pad so tap offsets never go negative
CHUNK = 3440                # LEAD + GRID + 12 tail, %16 == 0 (DoubleRow step)
NROW_GROUPS = 7
ROWS_PER_GROUP = H // NROW_GROUPS   # 8
FREE = ROWS_PER_GROUP * WP          # 464 <= 512 (one PSUM bank, fp32)
CI_CHUNKS = CIN // 128
CO_CHUNKS = COUT // 128

F32 = mybir.dt.float32
FP8 = mybir.dt.float8e4
ALU = mybir.AluOpType
AF = mybir.ActivationFunctionType
DR = mybir.MatmulPerfMode.DoubleRow

# tap groups for the weight-transpose drains: 4+4+1 blocks per 512-f32 PSUM bank
TAP_GROUPS = [(0, 4), (4, 4), (8, 1)]


def _body(tc, x_d, w_d, b_d, o_d, repeats=1, parts="full"):
    nc = tc.nc

    from contextlib import ExitStack
    ctx = ExitStack()
    with ctx:
        const_pool = ctx.enter_context(tc.tile_pool(name="const", bufs=1))
        wd_pool = ctx.enter_context(tc.tile_pool(name="wd", bufs=1))
        wsb_pool = ctx.enter_context(tc.tile_pool(name="wsb", bufs=1))
        xpad_pool = ctx.enter_context(tc.tile_pool(name="xpad", bufs=1))
        xin_pool = ctx.enter_context(tc.tile_pool(name="xin", bufs=3))
        out_pool = ctx.enter_context(tc.tile_pool(name="outs", bufs=2))

        ident = const_pool.tile([128, 128], F32, tag="ident", name="ident")
        masks.make_identity(nc, ident[:])

        bias_sb = const_pool.tile([128, CO_CHUNKS], F32, tag="bias",
                                  name="bias_sb")

        o_d3 = [[o_d[n, cc * 128:(cc + 1) * 128].rearrange("c h w -> c (h w)")
                 for cc in range(CO_CHUNKS)] for n in range(NPC)]

        for rep in range(repeats):
            # ---- weight phase: contiguous DMAs + on-chip transpose ----
            # wsb[cc]: [o_local=128, (i, tap)] — HBM-contiguous (i, tap) on
            # the free axis, so this is 128 descriptors of 9216B per chunk.
            # One tile per co-chunk (so cc0's transposes depend only on cc0's
            # DMA), issued on the ACT HWDGE ring (nc.scalar) while the SP
            # ring streams x.
            w_src = w_d.rearrange("(oc p) i kh kw -> p oc (i kh kw)", p=128)
            wsb = []
            for cc in range(CO_CHUNKS):
                wt = wsb_pool.tile([128, CIN * KS * KS], F32,
                                   tag=f"wsb{cc}", name=f"wsb{rep}_{cc}")
                # halves by ci-chunk: the two=0 transposes only wait for the
                # first half, shortening the weight-phase critical chain
                half = 128 * KS * KS
                nc.scalar.dma_start(wt[:, :half], w_src[:, cc, :half])
                nc.scalar.dma_start(wt[:, half:], w_src[:, cc, half:])
                wsb.append(wt)
            if rep == 0:
                nc.scalar.dma_start(bias_sb[:],
                                    b_d.rearrange("(c p) -> p c", p=128))
            wviews = [t[:].rearrange("p (i t) -> p i t", t=KS * KS)
                      for t in wsb]

            # wd8[cc]: [128 ci_local, 9*256] fp8, free idx = tap*256 + two*128
            # + co, values (w>=0)-0.5 in {+-0.5}. (lhsT slice per tap:
            # [k][two][m], steps [128, 1] — DoubleRow pairing contracts
            # (k, two) elementwise on both operands.)
            wd8 = []
            for cc in range(CO_CHUNKS):
                wt = wd_pool.tile([128, KS * KS * 256], FP8, tag=f"wd{cc}",
                                  name=f"wd8_{rep}_{cc}")
                wd8.append(wt)

            xpall = xpad_pool.tile([128, NPC * CI_CHUNKS * CHUNK], FP8,
                                   tag="xpall", name=f"xpall{rep}")
            xg4 = xpall[:].rearrange("c (n t s) -> c n t s",
                                     t=CI_CHUNKS, s=CHUNK)
            hh = H // 2

            def emit_weight_cc(cc, wtpsum):
                wt3 = wd8[cc][:].rearrange("k (t x) -> k t x", t=KS * KS)
                for two in range(CI_CHUNKS):
                    for g, (t0, tn) in enumerate(TAP_GROUPS):
                        pt = wtpsum.tile([128, 512], F32, tag="wtp",
                                         name=f"wtp{rep}_{cc}_{two}_{g}")
                        for j in range(tn):
                            nc.tensor.transpose(
                                pt[:, j * 128:(j + 1) * 128],
                                wviews[cc][:, two * 128:(two + 1) * 128,
                                           t0 + j],
                                ident[:])
                        # drain + binarize: {+-0.5} fp8, scattered to
                        # [tap][two][co] (dst strides: tap 256, co 1)
                        nc.vector.tensor_scalar(
                            wt3[:, t0:t0 + tn, two * 128:(two + 1) * 128],
                            pt[:, :tn * 128].rearrange(
                                "k (t x) -> k t x", x=128),
                            0.0, 0.5, op0=ALU.is_ge, op1=ALU.subtract)

            def emit_memsets():
                # borders of all 8 padded grids zeroed with 6 multi-grid
                # strided memsets (disjoint from the binarize interiors,
                # so they run concurrently)
                xg = xpall[:].rearrange("c (g s) -> c g s", s=CHUNK)
                nc.gpsimd.memset(xg[:, :, 0:LEAD], 0.0)
                nc.gpsimd.memset(xg[:, :, LEAD + GRID:CHUNK], 0.0)
                xgrid = xg[:, :, LEAD:LEAD + GRID] \
                    .rearrange("c g (h w) -> c g h w", w=WP)
                nc.gpsimd.memset(xgrid[:, :, 0:1, :], 0.0)
                nc.gpsimd.memset(xgrid[:, :, HP - 1:HP, :], 0.0)
                nc.gpsimd.memset(xgrid[:, :, 1:HP - 1, 0:1], 0.0)
                nc.gpsimd.memset(xgrid[:, :, 1:HP - 1, WP - 1:WP], 0.0)

            def emit_input(n):
                # x loads and binarize split in row-halves, top halves of
                # both ci-chunks first: the first matmul group only reads
                # rows 0-9, so it starts after ~half of image 0 has landed.
                # Image 0 binarizes on the (idle-until-outputs) GPSIMD so
                # DVE's early queue only carries the weight drains.
                eng = nc.gpsimd if n == 0 else nc.vector
                xraws = [xin_pool.tile([128, H * W], F32, tag="xraw",
                                       name=f"xraw{rep}_{n}_{two}")
                         for two in range(CI_CHUNKS)]
                for h in range(2):
                    for two in range(CI_CHUNKS):
                        xr_in = xraws[two][:].rearrange("c (h w) -> c h w",
                                                        w=W)
                        nc.sync.dma_start(
                            xr_in[:, h * hh:(h + 1) * hh],
                            x_d[n, two * 128:(two + 1) * 128,
                                h * hh:(h + 1) * hh])
                        xg_in = xg4[:, n, two, LEAD:LEAD + GRID] \
                            .rearrange("c (h w) -> c h w", w=WP)
                        eng.tensor_scalar(
                            xg_in[:, 1 + h * hh:1 + (h + 1) * hh, 1:W + 1],
                            xr_in[:, h * hh:(h + 1) * hh],
                            0.0, 0.5, op0=ALU.is_ge, op1=ALU.subtract)

            # HAM warmup: the PE is idle until the first weight DMA lands
            # (~4us) and would then run its first ~3.4us of real work at
            # 1.2 GHz (cold clock-gate). Dummy identity matmuls during the
            # DMA wait release the throttle before the transposes start.
            # (Transpose-mode matmuls don't count as PE-busy for HAM.)
            if rep == 0:
                with tc.tile_pool(name="warm", bufs=1, space="PSUM") as wp:
                    warm = wp.tile([128, 128], F32, tag="warm", name="warm")
                    for i in range(8):
                        nc.tensor.matmul(warm[:], ident[:], ident[:],
                                         start=True, stop=True)

            # emission order = scheduling priority: cc0 weights, then the
            # first image, then cc1 weights, then the remaining images —
            # so DVE's early queue isn't blocked by cc1's weight drains
            # (which wait on the second weight DMA) ahead of image-0
            # binarize work
            n_inputs = NPC if parts not in ("mmonly", "mmraw", "mmraweven", "mmrawsame") else 0
            with tc.tile_pool(name="wtp", bufs=2, space="PSUM") as wtpsum:
                emit_weight_cc(0, wtpsum)
                emit_memsets()
                if n_inputs:
                    emit_input(0)
                emit_weight_cc(1, wtpsum)
                for n in range(1, n_inputs):
                    emit_input(n)
            xp = [xpall[:, n * CI_CHUNKS * CHUNK:(n + 1) * CI_CHUNKS * CHUNK]
                  for n in range(NPC)]

            # ---- conv phase ----
            # per-row-group PSUM tiles rotating through all 8 banks: group
            # g+1's first matmul into a bank only waits for a drain from
            # ~1.5 groups earlier, so TensorE never stalls on drains
            with tc.tile_pool(name="cpsum", bufs=8, space="PSUM") as cpsum:
                ngroups = NPC * CO_CHUNKS if parts != "nomm" else 0
                for gi in range(ngroups):
                    n, cc = divmod(gi, CO_CHUNKS)
                    pps = [cpsum.tile([128, 512], F32, tag="cps",
                                      name=f"cps{rep}_{cc}_{n}_{rg}")
                           for rg in range(NROW_GROUPS)]
                    # rg-outer / tap-inner: consecutive matmuls stream
                    # 98%-overlapping rhs spans (offsets +-1, +-58), hitting
                    # the HW span-reuse fast path (measured: re-streaming an
                    # identical span is ~1.4x faster than a fresh one; LDW
                    # is emitted per-matmul either way, so tap-inner loses
                    # nothing on the weight side)
                    for rg in range(NROW_GROUPS):
                        for kpos in range(KS * KS):
                            kh, kw = divmod(kpos, KS)
                            lhsT = wd8[cc][:, kpos * 256:(kpos + 1) * 256] \
                                .rearrange("k (two m) -> k two m", two=2)
                            off = (LEAD + WP + rg * FREE
                                   + (kh - 1) * WP + (kw - 1))
                            if parts == "mmraweven":
                                # timing-only probe: 16B-align every rhs
                                # span to test DR column-fetch alignment
                                off &= ~15
                            elif parts == "mmrawsame":
                                # timing-only probe: all 9 taps re-stream
                                # ONE identical (misaligned-base) span per
                                # rg — discriminates span-reuse vs
                                # alignment as the mmraweven fast path
                                off = LEAD + WP + rg * FREE
                            rhs = xp[n].rearrange(
                                "k (two s) -> k two s",
                                s=CHUNK)[:, :, off:off + FREE]
                            nc.tensor.matmul(
                                pps[rg][:, :FREE], lhsT,
                                rhs, start=(kpos == 0),
                                stop=(kpos == KS * KS - 1),
                                perf_mode=DR)
                    ob = out_pool.tile([128, NROW_GROUPS * ROWS_PER_GROUP * W],
                                       F32, tag="ob",
                                       name=f"ob{rep}_{cc}_{n}")
                    # per-row-group drains (x4 restores the +-0.25 products),
                    # alternating ACT/DVE so the serial drain tail halves
                    # (mmraw/nodrain ablations: 8-col token drains that still
                    # read every PSUM bank so DCE keeps the matmuls)
                    ncol = W if parts not in ("mmraw", "mmraweven", "mmrawsame", "nodrain") else 8
                    for rg in range(NROW_GROUPS):
                        drain_in = pps[rg][:, :FREE] \
                            .rearrange("m (r c) -> m r c", c=WP
                                       )[:, :, 1:ncol + 1]
                        drain_out = ob[:].rearrange(
                            "m (g r c) -> m g r c", g=NROW_GROUPS, c=W
                            )[:, rg, :, :ncol]
                        if rg % 2 == 0:
                            nc.scalar.activation(
                                drain_out, drain_in,
                                AF.Identity, bias=bias_sb[:, cc:cc + 1],
                                scale=4.0)
                        else:
                            nc.vector.tensor_scalar(
                                drain_out, drain_in,
                                4.0, bias_sb[:, cc:cc + 1],
                                op0=ALU.mult, op1=ALU.add)
                    # outputs ride the idle SWDGE/Pool path so they never
                    # contend with the SP ring streaming x; the last group
                    # is split so its early quarters overlap the final drains
                    ob_g = ob[:].rearrange("m (g s) -> m g s", g=NROW_GROUPS)
                    od_g = o_d3[n][cc].rearrange("c (g s) -> c g s",
                                                 g=NROW_GROUPS)
                    if parts in ("noout", "mmraw", "mmraweven", "mmrawsame", "nodrain"):
                        # tiny consumer keeps drains/MMs live through DCE
                        nc.gpsimd.dma_start(od_g[:, 0, :64], ob_g[:, 0, :64])
                    elif gi == ngroups - 1:
                        # both HWDGE rings are idle by now; alternate the
                        # quarters so the tail transfer time halves
                        for qi, (lo, hi) in enumerate(
                                ((0, 2), (2, 4), (4, 6), (6, 7))):
                            eng = nc.sync if qi % 2 == 0 else nc.scalar
                            eng.dma_start(od_g[:, lo:hi], ob_g[:, lo:hi])
                    else:
                        nc.gpsimd.dma_start(o_d3[n][cc], ob[:])


_nc_cache = {}


def _get_nc(repeats=1, parts="full"):
    key = (repeats, parts)
    if key not in _nc_cache:
        nc = bacc.Bacc("TRN2", debug=False)
        x_d = nc.dram_tensor("x", [NPC, CIN, H, W], F32, kind="ExternalInput").ap()
        w_d = nc.dram_tensor("w", [COUT, CIN, KS, KS], F32,
                             kind="ExternalInput").ap()
        b_d = nc.dram_tensor("b", [COUT], F32, kind="ExternalInput").ap()
        o_d = nc.dram_tensor("out", [NPC, COUT, H, W], F32,
                             kind="ExternalOutput").ap()
        with tile.TileContext(nc) as tc:
            _body(tc, x_d, w_d, b_d, o_d, repeats=repeats, parts=parts)
        nc.compile()
        _nc_cache[key] = nc
    return _nc_cache[key]


# ---- persistent PJRT runner ---------------------------------------------
# bass_utils.run_bass_kernel_spmd builds a FRESH jax.jit closure per call, so
# every invocation re-lowers the module, re-ships the NEFF through axon, and
# re-loads it onto all 8 devices — seconds of overhead per call that has
# nothing to do with device execution. Here the jitted executable (and hence
# the loaded NEFF) is cached: the first call pays compile+load once, later
# calls only dispatch.

_runner_cache = {}


def _get_runner(repeats=1, parts="full"):
    key = (repeats, parts)
    if key in _runner_cache:
        return _runner_cache[key]

    import jax
    from jax.sharding import Mesh, PartitionSpec, NamedSharding
    from jax.experimental.shard_map import shard_map
    from concourse import bass2jax, mybir as mb

    nc = _get_nc(repeats, parts)
    bass2jax.install_neuronx_cc_hook()

    partition_name = (nc.partition_id_tensor.name
                      if nc.partition_id_tensor else None)
    in_names, out_names, out_avals, zero_shapes = [], [], [], []
    for alloc in nc.m.functions[0].allocations:
        if not isinstance(alloc, mb.MemoryLocationSet):
            continue
        name = alloc.memorylocations[0].name
        if alloc.kind == "ExternalInput":
            if name != partition_name:
                in_names.append(name)
        elif alloc.kind == "ExternalOutput":
            out_names.append(name)
            shape = tuple(alloc.tensor_shape)
            dtype = mb.dt.np(alloc.dtype)
            out_avals.append(jax.core.ShapedArray(shape, dtype))
            zero_shapes.append((shape, dtype))
    n_params = len(in_names)
    n_outs = len(out_names)
    in_names = in_names + out_names
    if partition_name is not None:
        in_names = in_names + [partition_name]

    def _body_fn(*args):
        operands = list(args)
        if partition_name is not None:
            operands.append(bass2jax.partition_id_tensor())
        outs = bass2jax._bass_exec_p.bind(
            *operands,
            out_avals=tuple(out_avals),
            in_names=tuple(in_names),
            out_names=tuple(out_names),
            lowering_input_output_aliases=(),
            sim_require_finite=True,
            sim_require_nnan=True,
            nc=nc,
        )
        return tuple(outs)

    devices = jax.devices()[:N_CORES]
    mesh = Mesh(np.asarray(devices), ("core",))
    sharding = NamedSharding(mesh, PartitionSpec("core"))
    donate = tuple(range(n_params, n_params + n_outs))
    fn = jax.jit(
        shard_map(
            _body_fn, mesh=mesh,
            in_specs=(PartitionSpec("core"),) * (n_params + n_outs),
            out_specs=(PartitionSpec("core"),) * n_outs,
            check_rep=False,
        ),
        donate_argnums=donate, keep_unused=True,
    )
    # On-device sharded zero buffers for the donated outputs (regenerated per
    # call — donation consumes them). No host->device traffic involved.
    import jax.numpy as jnp
    zeros_fn = jax.jit(
        lambda: tuple(jnp.zeros((N_CORES * s[0],) + s[1:], d)
                      for s, d in zero_shapes),
        out_shardings=(sharding,) * n_outs,
    )
    runner = {"fn": fn, "zeros_fn": zeros_fn, "sharding": sharding,
              "n_params": n_params, "in_order": in_names[:n_params]}
    _runner_cache[key] = runner
    return runner


def _device_inputs(inputs, runner):
    """Concat per-core shards on axis 0 and put on the 8 devices."""
    import jax
    x, w, b = inputs["x"], inputs["w"], inputs["b"]
    full = {
        "x": np.ascontiguousarray(x, dtype=np.float32),
        "w": np.ascontiguousarray(
            np.broadcast_to(w, (N_CORES,) + tuple(w.shape)).reshape(
                (N_CORES * w.shape[0],) + tuple(w.shape[1:])),
            dtype=np.float32),
        "b": np.ascontiguousarray(
            np.broadcast_to(b, (N_CORES,) + tuple(b.shape)).reshape(
                (N_CORES * b.shape[0],) + tuple(b.shape[1:])),
            dtype=np.float32),
    }
    arrs = [full[name] for name in runner["in_order"]]
    return jax.device_put(arrs, [runner["sharding"]] * len(arrs))


def _exec(runner, dev_in):
    outs = runner["fn"](*dev_in, *runner["zeros_fn"]())
    return outs


def _run(inputs, repeats=1, parts="full", **kwargs):
    x = inputs["x"]
    assert x.shape == (N, CIN, H, W), x.shape
    runner = _get_runner(repeats, parts)
    dev_in = _device_inputs(inputs, runner)
    outs = _exec(runner, dev_in)
    out = np.asarray(outs[0])

    class _Res:
        exec_time_ns = None
        instructions_and_trace = None
    return out, _Res()


def kernel(**inputs) -> np.ndarray:
    out, _ = _run(inputs)
    return out

